# revision 31
# baseline (speedup 1.0000x reference)
"""ABCDense (ShiftedSteSign 3-estimator binary dense) Trainium2 kernel.

Math:
    xq   = sign(x)                      [N, D]   (+1 for x >= 0)
    beta = mean(|x|, axis=-1)           [N]
    out  = sum_e (xq @ sign(k_e)) * (beta[:,None] * a_e[None,:])

Folded form used here (column scaling commutes with the GEMM):
    W    = sum_e sign(k_e) * (a_e / D)[None,:]        [D, U]  (bf16)
    out  = sum_abs_x[:,None] * (xq @ W)

Distribution: pure data-parallel over the N=32768 token axis across 8
cores (4096 rows each); k/a replicated (no collectives: any cross-core
sync point adds the cores' launch skew to the measured span).

Pipeline structure (software-pipelined emission, two staggered leads):
  lead LOOK: x load (Sync ring) -> DVE |x|=max(-x,x) fused row-sum
             (beta)
  lead TL:   8 PE transposes (f32, transpose_mode) into PSUM ->
             ACT evacuates with fused Sign straight to bf16 xqT
  lead 0:    16 bf16 matmuls (h-outer, one PSUM bank per half) ->
             PSUM evac with fused per-partition beta scale, split
             DVE(h0)/ACT(h1) -> out store on the gpsimd SWDGE ring.
Out stores go via gpsimd so their completions count on the DMASW sem
lanes: on the 8 shared round-robin DMAHW lanes a wait for "my x load
done" would otherwise transitively require the previous tiles'
compute-gated out stores to finish (lane aliasing), chaining every MM
block to the previous tile's evacuation.
The +-1 GEMM is exact in bf16; PSUM accumulates exact integers.
"""

import numpy as np

import concourse.mybir as mybir
from concourse import bacc, tile
from concourse.bass_utils import run_bass_kernel_spmd
from concourse.masks import make_identity

F32 = mybir.dt.float32
BF16 = mybir.dt.bfloat16
AF = mybir.ActivationFunctionType
ALU = mybir.AluOpType

N, D, U, E = 32768, 1024, 1024, 3
NCORES = 8
NS = N // NCORES            # 4096 rows per core
P = 128                     # partitions
DC = D // P                 # 8 d-chunks
NT = NS // P                # 32 n-tiles of 128 tokens
UH = 512                    # u half (one PSUM bank per matmul)
LOOK = 10                   # x/beta lead (tiles)
TL = 2                      # transpose lead (tiles; small so prologue
                            # transposes never head-of-line-block the
                            # first matmuls in the in-order PE FIFO)


def build_nc():
    nc = bacc.Bacc(
        "TRN2",
        target_bir_lowering=False,
        debug=False,
        num_devices=NCORES,
    )

    # --- DRAM parameters (per-core shapes) ---
    x_d = nc.dram_tensor("x", [NS, D], F32, kind="ExternalInput")
    ks = [nc.dram_tensor(f"k{e}", [D, U], F32, kind="ExternalInput") for e in range(E)]
    as_ = [nc.dram_tensor(f"a{e}", [1, U], F32, kind="ExternalInput") for e in range(E)]
    out_d = nc.dram_tensor("out", [NS, U], BF16, kind="ExternalOutput")

    with tile.TileContext(nc) as tc:
        with (
            tc.tile_pool(name="const", bufs=1) as const,
            tc.tile_pool(name="kstage", bufs=3) as kpool,
            tc.tile_pool(name="xin", bufs=LOOK + 1) as xpool,
            tc.tile_pool(name="scr", bufs=2) as scrpool,
            tc.tile_pool(name="xqt", bufs=TL + 2) as xqtpool,
            tc.tile_pool(name="osb", bufs=3) as opool,
            tc.tile_pool(name="psT", bufs=2, space="PSUM") as psumT,
            tc.tile_pool(name="psM", bufs=6, space="PSUM") as psumM,
        ):
            ident = const.tile([P, P], F32)
            make_identity(nc, ident[:])

            beta_cols = const.tile([P, NT], F32)

            # a loads first (tiny; the folds need a_bcast early)
            a_fs = []
            for e in range(E):
                a_f = const.tile([1, U], F32, tag=f"a_f{e}")
                nc.sync.dma_start(out=a_f[0:1, :], in_=as_[e][:, :])
                a_fs.append(a_f)

            # k loads next, in d-halves, split across BOTH HWDGE rings
            # (Sync + ACT): each ring transfers serially at ~400GB/s, so
            # the split halves the 12.6MB k-load ramp. The ACT-ring
            # issues are emitted before any ACT compute, so they fire
            # back-to-back and never interleave with the sign chain.
            kes = {}
            xts = {}
            for dh in range(2):
                for e in range(E):
                    ke = kpool.tile([P, DC // 2, U], F32, tag="ke")
                    nc.sync.dma_start(
                        out=ke[:],
                        in_=ks[e][dh * (D // 2):(dh + 1) * (D // 2), :]
                        .rearrange("(c p) u -> p c u", p=P),
                    )
                    kes[(e, dh)] = ke
                if dh == 0:
                    # first x tiles between the k halves: the prologue
                    # transposes get PE work ~15us earlier while the
                    # second half of k still loads
                    for t in range(3):
                        x_t = xpool.tile([P, D], F32, tag="xt")
                        nc.sync.dma_start(
                            out=x_t[:], in_=x_d[t * P:(t + 1) * P, :]
                        )
                        xts[t] = x_t

            a_bcast = []
            for e in range(E):
                a_b = const.tile([1, U], BF16, tag=f"a_b{e}")
                nc.vector.tensor_scalar(
                    a_b[0:1, :], a_fs[e][0:1, :], 1.0 / D, None, op0=ALU.mult
                )
                a_full = const.tile([P, U], BF16, tag=f"a_full{e}")
                nc.gpsimd.partition_broadcast(a_full[:], a_b[0:1, :])
                a_bcast.append(a_full)

            # ---------- W = sum_e sign(k_e) * a_e / D  (bf16, [d-part, c, u]) ----------
            # q-major across estimators so W chunks complete in c-order
            # and the first matmuls start as early as possible
            W = const.tile([P, DC, U], BF16)
            for q in range(4):
                dh = q // 2
                for e in range(E):
                    ke = kes[(e, dh)]
                    qq = q % 2
                    s_q = kpool.tile([P, 2, U], BF16, tag="se")
                    nc.scalar.activation(
                        s_q[:], ke[:, qq * 2:(qq + 1) * 2, :], AF.Sign,
                    )
                    for cc in range(2):
                        c = q * 2 + cc
                        if e == 0:
                            nc.vector.tensor_tensor(
                                W[:, c, :], s_q[:, cc, :], a_bcast[e][:],
                                op=ALU.mult
                            )
                        else:
                            tmp = kpool.tile([P, U], BF16, tag="tmp")
                            nc.vector.tensor_tensor(
                                tmp[:], s_q[:, cc, :], a_bcast[e][:],
                                op=ALU.mult
                            )
                            nc.vector.tensor_tensor(
                                W[:, c, :], W[:, c, :], tmp[:], op=ALU.add
                            )

            # ---------- software-pipelined main loop ----------
            def prep_x(t):
                if t in xts:
                    x_t = xts[t]
                else:
                    x_t = xpool.tile([P, D], F32, tag="xt")
                    nc.sync.dma_start(out=x_t[:], in_=x_d[t * P:(t + 1) * P, :])
                # beta: |x|=max(-x,x) with fused row-sum on DVE (raw sum;
                # the 1/D of beta is folded into W)
                scratch = scrpool.tile([P, D], F32, tag="scratch")
                nc.vector.scalar_tensor_tensor(
                    scratch[:], x_t[:], -1.0, x_t[:],
                    op0=ALU.mult, op1=ALU.max,
                    accum_out=beta_cols[:, t:t + 1],
                )
                return x_t

            def transpose(x_t):
                # PE transposes (f32, full-rate transpose_mode) into PSUM;
                # ACT evacuates with fused Sign straight to bf16 xqT
                xqT = xqtpool.tile([P, DC, P], BF16, tag="xqT")
                for half in range(2):
                    psT = psumT.tile([P, 4 * P], F32, tag="psT")
                    for j in range(4):
                        c = 4 * half + j
                        nc.tensor.transpose(
                            psT[:, j * P:(j + 1) * P],
                            x_t[:, c * P:(c + 1) * P],
                            ident[:],
                        )
                    nc.scalar.activation(
                        xqT[:, 4 * half:4 * half + 4, :], psT[:], AF.Sign
                    )
                return xqT

            xts = {}
            xqTs = {}
            for t in range(LOOK):
                xts[t] = prep_x(t)
            for t in range(TL):
                xqTs[t] = transpose(xts[t])

            for t in range(NT):
                xqT = xqTs.pop(t)
                xts.pop(t)
                # c-outer/h-inner: each xqT chunk is loaded as stationary
                # once and streams both u-halves (8 LDWEIGHTS per tile
                # instead of 16), and the c-order still matches the
                # q-major W-chunk readiness during the ramp
                ps0 = psumM.tile([P, UH], F32, tag="ps")
                ps1 = psumM.tile([P, UH], F32, tag="ps")
                ps = [ps0, ps1]
                for c in range(DC):
                    for h in range(2):
                        nc.tensor.matmul(
                            ps[h][:],
                            xqT[:, c, :],
                            W[:, c, h * UH:(h + 1) * UH],
                            start=(c == 0), stop=(c == DC - 1),
                        )
                # split PSUM evacuation with fused per-partition beta
                # scale: h0 on DVE, h1 on ACT (bf16 out)
                osb = opool.tile([P, U], BF16, tag="osb")
                bcol = beta_cols[:, t:t + 1]
                nc.vector.tensor_scalar(
                    osb[:, 0:UH], ps[0][:], bcol, None, op0=ALU.mult
                )
                nc.scalar.activation(
                    osb[:, UH:U], ps[1][:], AF.Copy, scale=bcol
                )
                # out store on the Sync ring: with PE-side transposes the
                # only DMAHW-lane waiters are the x-load consumers, which
                # run several tiles ahead of the aliased out stores, so the
                # lane aliasing costs nothing (and the SWDGE path's slow
                # drain added ~12us of tail)
                nc.sync.dma_start(out=out_d[t * P:(t + 1) * P, :], in_=osb[:])
                if t + TL < NT:
                    xqTs[t + TL] = transpose(xts[t + TL])
                if t + LOOK < NT:
                    xts[t + LOOK] = prep_x(t + LOOK)

    nc.compile()
    return nc


_CACHE = {}


def _get_nc():
    if "nc" not in _CACHE:
        _CACHE["nc"] = build_nc()
    return _CACHE["nc"]


def make_in_maps(x, k0, k1, k2, a0, a1, a2):
    x = np.ascontiguousarray(x, dtype=np.float32)
    ks = [np.ascontiguousarray(k, dtype=np.float32) for k in (k0, k1, k2)]
    as_ = [np.ascontiguousarray(a, dtype=np.float32).reshape(1, U) for a in (a0, a1, a2)]
    in_maps = []
    for i in range(NCORES):
        shard = np.ascontiguousarray(x[i * NS:(i + 1) * NS])
        in_maps.append({
            "x": shard,
            **{f"k{e}": ks[e] for e in range(E)},
            **{f"a{e}": as_[e] for e in range(E)},
        })
    return in_maps


def run_sharded(x, k0, k1, k2, a0, a1, a2, trace=False, **kw):
    nc = _get_nc()
    in_maps = make_in_maps(x, k0, k1, k2, a0, a1, a2)
    res = run_bass_kernel_spmd(nc, in_maps, list(range(NCORES)), trace=trace, **kw)
    out = np.concatenate(
        [np.asarray(res.results[i]["out"]).astype(np.float32) for i in range(NCORES)],
        axis=0,
    )
    return out, res


def kernel(x, k0, k1, k2, a0, a1, a2):
    out, _ = run_sharded(x, k0, k1, k2, a0, a1, a2, trace=False)
    return out


# revision 32
# speedup vs baseline: 1.0478x; 1.0478x over previous
"""ABCDense (ShiftedSteSign 3-estimator binary dense) Trainium2 kernel.

Math:
    xq   = sign(x)                      [N, D]   (+1 for x >= 0)
    beta = mean(|x|, axis=-1)           [N]
    out  = sum_e (xq @ sign(k_e)) * (beta[:,None] * a_e[None,:])

Folded form used here (column scaling commutes with the GEMM):
    W    = sum_e sign(k_e) * (a_e / D)[None,:]        [D, U]  (bf16)
    out  = sum_abs_x[:,None] * (xq @ W)

Distribution: pure data-parallel over the N=32768 token axis across 8
cores (4096 rows each); k/a replicated (no collectives: any cross-core
sync point adds the cores' launch skew to the measured span).

Pipeline structure (software-pipelined emission, two staggered leads):
  lead LOOK: x load (Sync ring) -> DVE |x|=max(-x,x) fused row-sum
             (beta)
  lead TL:   8 PE transposes (f32, transpose_mode) into PSUM ->
             ACT evacuates with fused Sign straight to bf16 xqT
  lead 0:    16 bf16 matmuls (h-outer, one PSUM bank per half) ->
             PSUM evac with fused per-partition beta scale, split
             DVE(h0)/ACT(h1) -> out store on the gpsimd SWDGE ring.
Out stores go via gpsimd so their completions count on the DMASW sem
lanes: on the 8 shared round-robin DMAHW lanes a wait for "my x load
done" would otherwise transitively require the previous tiles'
compute-gated out stores to finish (lane aliasing), chaining every MM
block to the previous tile's evacuation.
The +-1 GEMM is exact in bf16; PSUM accumulates exact integers.
"""

import numpy as np

import concourse.mybir as mybir
from concourse import bacc, tile
from concourse.bass_utils import run_bass_kernel_spmd
from concourse.masks import make_identity

F32 = mybir.dt.float32
BF16 = mybir.dt.bfloat16
AF = mybir.ActivationFunctionType
ALU = mybir.AluOpType

N, D, U, E = 32768, 1024, 1024, 3
NCORES = 8
NS = N // NCORES            # 4096 rows per core
P = 128                     # partitions
DC = D // P                 # 8 d-chunks
NT = NS // P                # 32 n-tiles of 128 tokens
UH = 512                    # u half (one PSUM bank per matmul)
LOOK = 10                   # x/beta lead (tiles)
TL = 2                      # transpose lead (tiles; small so prologue
                            # transposes never head-of-line-block the
                            # first matmuls in the in-order PE FIFO)


def build_nc():
    nc = bacc.Bacc(
        "TRN2",
        target_bir_lowering=False,
        debug=False,
        num_devices=NCORES,
    )

    # --- DRAM parameters (per-core shapes) ---
    x_d = nc.dram_tensor("x", [NS, D], F32, kind="ExternalInput")
    ks = [nc.dram_tensor(f"k{e}", [D, U], F32, kind="ExternalInput") for e in range(E)]
    as_ = [nc.dram_tensor(f"a{e}", [1, U], F32, kind="ExternalInput") for e in range(E)]
    out_d = nc.dram_tensor("out", [NS, U], BF16, kind="ExternalOutput")

    with tile.TileContext(nc) as tc:
        with (
            tc.tile_pool(name="const", bufs=1) as const,
            tc.tile_pool(name="kstage", bufs=3) as kpool,
            tc.tile_pool(name="xin", bufs=LOOK + 1) as xpool,
            tc.tile_pool(name="scr", bufs=2) as scrpool,
            tc.tile_pool(name="xqt", bufs=TL + 2) as xqtpool,
            tc.tile_pool(name="osb", bufs=3) as opool,
            tc.tile_pool(name="psT", bufs=2, space="PSUM") as psumT,
            tc.tile_pool(name="psM", bufs=6, space="PSUM") as psumM,
        ):
            ident = const.tile([P, P], F32)
            make_identity(nc, ident[:])

            beta_cols = const.tile([P, NT], F32)

            # a loads first (tiny; the folds need a_bcast early)
            a_fs = []
            for e in range(E):
                a_f = const.tile([1, U], F32, tag=f"a_f{e}")
                nc.sync.dma_start(out=a_f[0:1, :], in_=as_[e][:, :])
                a_fs.append(a_f)

            # k loads next, in d-halves, split across BOTH HWDGE rings
            # (Sync + ACT): each ring transfers serially at ~400GB/s, so
            # the split halves the 12.6MB k-load ramp. The ACT-ring
            # issues are emitted before any ACT compute, so they fire
            # back-to-back and never interleave with the sign chain.
            kes = {}
            xts = {}
            for dh in range(2):
                for e in range(E):
                    ke = kpool.tile([P, DC // 2, U], F32, tag="ke")
                    nc.sync.dma_start(
                        out=ke[:],
                        in_=ks[e][dh * (D // 2):(dh + 1) * (D // 2), :]
                        .rearrange("(c p) u -> p c u", p=P),
                    )
                    kes[(e, dh)] = ke

            a_bcast = []
            for e in range(E):
                a_b = const.tile([1, U], BF16, tag=f"a_b{e}")
                nc.vector.tensor_scalar(
                    a_b[0:1, :], a_fs[e][0:1, :], 1.0 / D, None, op0=ALU.mult
                )
                a_full = const.tile([P, U], BF16, tag=f"a_full{e}")
                nc.gpsimd.partition_broadcast(a_full[:], a_b[0:1, :])
                a_bcast.append(a_full)

            # ---------- W = sum_e sign(k_e) * a_e / D  (bf16, [d-part, c, u]) ----------
            # q-major across estimators so W chunks complete in c-order
            # and the first matmuls start as early as possible
            W = const.tile([P, DC, U], BF16)
            for q in range(4):
                dh = q // 2
                for e in range(E):
                    ke = kes[(e, dh)]
                    qq = q % 2
                    s_q = kpool.tile([P, 2, U], BF16, tag="se")
                    nc.scalar.activation(
                        s_q[:], ke[:, qq * 2:(qq + 1) * 2, :], AF.Sign,
                    )
                    for cc in range(2):
                        c = q * 2 + cc
                        if e == 0:
                            nc.vector.tensor_tensor(
                                W[:, c, :], s_q[:, cc, :], a_bcast[e][:],
                                op=ALU.mult
                            )
                        else:
                            tmp = kpool.tile([P, U], BF16, tag="tmp")
                            nc.vector.tensor_tensor(
                                tmp[:], s_q[:, cc, :], a_bcast[e][:],
                                op=ALU.mult
                            )
                            nc.vector.tensor_tensor(
                                W[:, c, :], W[:, c, :], tmp[:], op=ALU.add
                            )

            # ---------- software-pipelined main loop ----------
            def prep_x(t):
                if t in xts:
                    x_t = xts[t]
                else:
                    x_t = xpool.tile([P, D], F32, tag="xt")
                    nc.sync.dma_start(out=x_t[:], in_=x_d[t * P:(t + 1) * P, :])
                # beta: |x|=max(-x,x) with fused row-sum on DVE (raw sum;
                # the 1/D of beta is folded into W)
                scratch = scrpool.tile([P, D], F32, tag="scratch")
                nc.vector.scalar_tensor_tensor(
                    scratch[:], x_t[:], -1.0, x_t[:],
                    op0=ALU.mult, op1=ALU.max,
                    accum_out=beta_cols[:, t:t + 1],
                )
                return x_t

            def transpose(x_t):
                # PE transposes (f32, full-rate transpose_mode) into PSUM;
                # ACT evacuates with fused Sign straight to bf16 xqT
                xqT = xqtpool.tile([P, DC, P], BF16, tag="xqT")
                for half in range(2):
                    psT = psumT.tile([P, 4 * P], F32, tag="psT")
                    for j in range(4):
                        c = 4 * half + j
                        nc.tensor.transpose(
                            psT[:, j * P:(j + 1) * P],
                            x_t[:, c * P:(c + 1) * P],
                            ident[:],
                        )
                    nc.scalar.activation(
                        xqT[:, 4 * half:4 * half + 4, :], psT[:], AF.Sign
                    )
                return xqT

            xts = {}
            xqTs = {}
            for t in range(LOOK):
                xts[t] = prep_x(t)
            for t in range(TL):
                xqTs[t] = transpose(xts[t])

            for t in range(NT):
                xqT = xqTs.pop(t)
                xts.pop(t)
                # c-outer/h-inner: each xqT chunk is loaded as stationary
                # once and streams both u-halves (8 LDWEIGHTS per tile
                # instead of 16), and the c-order still matches the
                # q-major W-chunk readiness during the ramp
                ps0 = psumM.tile([P, UH], F32, tag="ps")
                ps1 = psumM.tile([P, UH], F32, tag="ps")
                ps = [ps0, ps1]
                for c in range(DC):
                    for h in range(2):
                        nc.tensor.matmul(
                            ps[h][:],
                            xqT[:, c, :],
                            W[:, c, h * UH:(h + 1) * UH],
                            start=(c == 0), stop=(c == DC - 1),
                        )
                # split PSUM evacuation with fused per-partition beta
                # scale: h0 on DVE, h1 on ACT (bf16 out)
                osb = opool.tile([P, U], BF16, tag="osb")
                bcol = beta_cols[:, t:t + 1]
                nc.vector.tensor_scalar(
                    osb[:, 0:UH], ps[0][:], bcol, None, op0=ALU.mult
                )
                nc.scalar.activation(
                    osb[:, UH:U], ps[1][:], AF.Copy, scale=bcol
                )
                # out store on the Sync ring: with PE-side transposes the
                # only DMAHW-lane waiters are the x-load consumers, which
                # run several tiles ahead of the aliased out stores, so the
                # lane aliasing costs nothing (and the SWDGE path's slow
                # drain added ~12us of tail)
                nc.sync.dma_start(out=out_d[t * P:(t + 1) * P, :], in_=osb[:])
                if t + TL < NT:
                    xqTs[t + TL] = transpose(xts[t + TL])
                if t + LOOK < NT:
                    xts[t + LOOK] = prep_x(t + LOOK)

    nc.compile()
    return nc


_CACHE = {}


def _get_nc():
    if "nc" not in _CACHE:
        _CACHE["nc"] = build_nc()
    return _CACHE["nc"]


def make_in_maps(x, k0, k1, k2, a0, a1, a2):
    x = np.ascontiguousarray(x, dtype=np.float32)
    ks = [np.ascontiguousarray(k, dtype=np.float32) for k in (k0, k1, k2)]
    as_ = [np.ascontiguousarray(a, dtype=np.float32).reshape(1, U) for a in (a0, a1, a2)]
    in_maps = []
    for i in range(NCORES):
        shard = np.ascontiguousarray(x[i * NS:(i + 1) * NS])
        in_maps.append({
            "x": shard,
            **{f"k{e}": ks[e] for e in range(E)},
            **{f"a{e}": as_[e] for e in range(E)},
        })
    return in_maps


def run_sharded(x, k0, k1, k2, a0, a1, a2, trace=False, **kw):
    nc = _get_nc()
    in_maps = make_in_maps(x, k0, k1, k2, a0, a1, a2)
    res = run_bass_kernel_spmd(nc, in_maps, list(range(NCORES)), trace=trace, **kw)
    out = np.concatenate(
        [np.asarray(res.results[i]["out"]).astype(np.float32) for i in range(NCORES)],
        axis=0,
    )
    return out, res


def kernel(x, k0, k1, k2, a0, a1, a2):
    out, _ = run_sharded(x, k0, k1, k2, a0, a1, a2, trace=False)
    return out
